# revision 24
# baseline (speedup 1.0000x reference)
"""Causal self-attention (B=4, T=2048, C=1024, H=16) on 8 trn2 NeuronCores.

Sharding: head-parallel. Each core owns 2 of the 16 heads (= 128 of the 1024
qkv channels). QKV projections are column-parallel, attention is fully local
per head, the output projection is row-parallel and the 8 partial outputs are
summed on the host (+ bp).

Device kernel (per core):
  - qT/kT/vT = W_slice^T-form matmuls over xT (fp32r, N=512) -> SBUF resident
  - S^T tiles [s=128, t<=512] = kT^T-slices x qT-slices (fp32r)
  - P^T = exp(S^T/8 + causal_additive_mask) on ACT (no-max softmax; scores
    are O(6) so exp cannot overflow), output bf16
  - P@V + softmax denominator in one bf16 matmul (ones column appended to V)
  - y = num * (1/den) on DVE, PE-transpose to yT, Wp matmul (fp32r) ->
    partial output [8192, 1024] in DRAM
"""

import os
import sys

for _p in ("/opt/trn_rl_repo", "/root/.axon_site/_ro/trn_rl_repo"):
    if os.path.isdir(_p) and _p not in sys.path:
        sys.path.insert(0, _p)

import numpy as np

B, T, C = 4, 2048, 1024
H, D = 16, 64
N_CORES = 8
CH = C // N_CORES          # qkv channels per core (= 2 heads x 64)
HPC = H // N_CORES         # heads per core
BT = B * T                 # 8192 tokens
P = 128
TSB = 512                  # token superblock
N_TSB = BT // TSB          # 16
SB_PER_B = T // TSB        # 4 superblocks per batch
ST_PER_B = T // P          # 16 s-tiles per batch

_RUNNER = None
CFG = {'pm': 4, 'pmq': 2, 'ptr': 1, 'ppv': 1, 'pt': 56, 'pipe': 1, 'cad': 3}


def _build_nc():
    import concourse.mybir as mybir
    import concourse.tile as tile
    from concourse import bacc

    f32 = mybir.dt.float32
    f32r = mybir.dt.float32r
    bf16 = mybir.dt.bfloat16
    MULT = mybir.AluOpType.mult
    EXP = mybir.ActivationFunctionType.Exp

    nc = bacc.Bacc("TRN2", target_bir_lowering=False, debug=False,
                   num_devices=N_CORES)

    xT = nc.dram_tensor("xT", [C, BT], f32r, kind="ExternalInput")
    wqT = nc.dram_tensor("wqT", [C, CH], f32r, kind="ExternalInput")
    wkT = nc.dram_tensor("wkT", [C, CH], f32r, kind="ExternalInput")
    wvT = nc.dram_tensor("wvT", [C, CH], f32r, kind="ExternalInput")
    bq = nc.dram_tensor("bq", [CH, 1], f32, kind="ExternalInput")
    bk = nc.dram_tensor("bk", [CH, 1], f32, kind="ExternalInput")
    bv = nc.dram_tensor("bv", [CH, 1], f32, kind="ExternalInput")
    wpT = nc.dram_tensor("wpT", [CH, C], f32r, kind="ExternalInput")
    tri01 = nc.dram_tensor("tri01", [P, P], bf16, kind="ExternalInput")
    ident = nc.dram_tensor("ident", [P, P], f32, kind="ExternalInput")
    outp = nc.dram_tensor("outp", [BT, C], f32, kind="ExternalOutput")

    with tile.TileContext(nc) as tc:
        with (
            tc.tile_pool(name="const", bufs=1) as const,
            tc.tile_pool(name="big", bufs=1) as big,
            tc.tile_pool(name="xp", bufs=2) as xp,
            tc.tile_pool(name="vt", bufs=2) as vtp,
            tc.tile_pool(name="pt", bufs=CFG['pt']) as ptp,
            tc.tile_pool(name="yp", bufs=3) as yp,
            tc.tile_pool(name="ytp", bufs=2) as ytp,
            tc.tile_pool(name="rp", bufs=4) as rp,
            tc.tile_pool(name="op", bufs=3) as op,
            tc.tile_pool(name="pm", bufs=CFG['pm'], space="PSUM") as pm,
            tc.tile_pool(name="pmq", bufs=max(CFG.get('pmq', 0), 1), space="PSUM") as pmq,
            tc.tile_pool(name="ptr", bufs=CFG['ptr'], space="PSUM") as ptr,
            tc.tile_pool(name="ppv", bufs=CFG['ppv'], space="PSUM") as ppv,
        ):
            # ---- constants (wq first: it gates the first matmul) ----
            wq_sb = const.tile([P, 8, CH], f32r, tag="wq")
            wk_sb = const.tile([P, 8, CH], f32r, tag="wk")
            wv_sb = const.tile([P, 8, CH], f32r, tag="wv")
            nc.sync.dma_start(wq_sb[:, 0:2, :],
                              wqT.ap().rearrange("(a p) m -> p a m", p=P)[:, 0:2, :])
            nc.sync.dma_start(wq_sb[:, 2:8, :],
                              wqT.ap().rearrange("(a p) m -> p a m", p=P)[:, 2:8, :])
            nc.sync.dma_start(wk_sb[:], wkT.ap().rearrange("(a p) m -> p a m", p=P))
            nc.sync.dma_start(wv_sb[:], wvT.ap().rearrange("(a p) m -> p a m", p=P))
            wp_sb = const.tile([P, C], f32r, tag="wp")
            bq_sb = const.tile([P, 1], f32, tag="bq")
            bk_sb = const.tile([P, 1], f32, tag="bk")
            bv_sb = const.tile([P, 1], f32, tag="bv")
            mask_sb = const.tile([P, P], bf16, tag="mask")
            ident_sb = const.tile([P, P], f32, tag="ident")
            nc.sync.dma_start(bq_sb[:], bq.ap())
            nc.sync.dma_start(bk_sb[:], bk.ap())
            nc.sync.dma_start(bv_sb[:], bv.ap())
            nc.sync.dma_start(wp_sb[:], wpT.ap())
            nc.sync.dma_start(mask_sb[:], tri01.ap())
            nc.sync.dma_start(ident_sb[:], ident.ap())

            # ---- resident activation buffers ----
            qT_sb = big.tile([P, BT], f32r, tag="qT")
            kT_sb = big.tile([P, BT], f32r, tag="kT")
            # v layout: per global s-tile idx (64 of them): 130 bf16 cols =
            # [head0 v (64) | ones | head1 v (64) | ones]
            v_sb = big.tile([P, 64 * 130], bf16, tag="v")
            nc.vector.memset(v_sb[:, 64::65], 1.0)

            def attn_S_units(b, sb, pts):
                """Generator: one (h, st) score tile + exp per unit."""
                t0 = b * T + sb * TSB
                nst = 4 * sb + 4
                for h in range(HPC):
                    hs = h * 64
                    for st in range(nst):
                        j0 = max(0, st - 4 * sb)
                        n0 = j0 * P
                        ps = pm.tile([P, TSB], f32, tag="mm", name="ps")
                        lhs = kT_sb[hs:hs + 64,
                                    (b * ST_PER_B + st) * P:(b * ST_PER_B + st + 1) * P]
                        rhs = qT_sb[hs:hs + 64, t0 + n0: t0 + TSB]
                        nc.tensor.matmul(
                            ps[:, n0:TSB], lhs, rhs,
                            start=True, stop=True)
                        ptile = ptp.tile([P, TSB], bf16, tag="pt", name="ptile")
                        nc.scalar.activation(
                            ptile[:, n0:TSB], ps[:, n0:TSB], EXP, scale=0.125)
                        if st >= 4 * sb:
                            nc.vector.tensor_tensor(
                                ptile[:, n0:n0 + P], ptile[:, n0:n0 + P],
                                mask_sb[:], MULT)
                        pts[(h, st)] = ptile
                        yield

            def attn_PV_units(b, sb, pts):
                """Generator: one output t-block (PV + div + transpose + Wp)."""
                for j in range(4):
                    y_t = yp.tile([P, P], f32, tag="y", name="y_t")
                    for h in range(HPC):
                        pv = ppv.tile([P, 65], f32, tag="pv", name="pv")
                        nv = 4 * sb + j + 1
                        for st in range(nv):
                            ptile = pts[(h, st)]
                            idx = b * ST_PER_B + st
                            nc.tensor.matmul(
                                pv[:],
                                ptile[:, j * P:(j + 1) * P],
                                v_sb[:, idx * 130 + h * 65: idx * 130 + h * 65 + 65],
                                start=(st == 0), stop=(st == nv - 1))
                        rec = rp.tile([P, 1], f32, tag="rec", name="rec")
                        nc.vector.reciprocal(rec[:], pv[:, 64:65])
                        nc.vector.tensor_scalar_mul(
                            y_t[:, h * 64:(h + 1) * 64], pv[:, 0:64], rec[:, 0:1])
                    pst = ptr.tile([P, P], f32, tag="tr", name="pst")
                    nc.tensor.transpose(pst[:], y_t[:], ident_sb[:])
                    yt_t = ytp.tile([P, P], f32r, tag="yt", name="yt_t")
                    nc.vector.tensor_copy(yt_t[:], pst[:])
                    r = (b * SB_PER_B + sb) * 4 + j
                    ot = op.tile([P, C], f32, tag="ot", name="ot")
                    for half in range(2):
                        wps = pm.tile([P, TSB], f32, tag="mm", name="wps")
                        nc.tensor.matmul(
                            wps[:], yt_t[:], wp_sb[:, half * TSB:(half + 1) * TSB],
                            start=True, stop=True)
                        nc.any.tensor_copy(
                            ot[:, half * TSB:(half + 1) * TSB], wps[:])
                    nc.sync.dma_start(outp.ap()[r * P:(r + 1) * P, :], ot[:])
                    yield

            def qkv_units(tsb):
                """Generator: one projection per unit (DMA rides with unit 0)."""
                xt = xp.tile([P, 8, TSB], f32r, tag="xt", name="xt")
                src = xT.ap().rearrange("(a p) t -> p a t", p=P)
                for c in range(4):  # chunked so matmuls start on first k-tiles
                    nc.sync.dma_start(
                        xt[:, 2 * c:2 * c + 2, :],
                        src[:, 2 * c:2 * c + 2, tsb * TSB:(tsb + 1) * TSB])
                for unit in qkv_superblock_body(tsb, xt):
                    yield

            def qkv_superblock_body(tsb, xt):
                qpool = pmq if CFG.get('pmq', 0) else pm
                for (w_sb, b_sb, kind) in (
                    (wq_sb, bq_sb, "q"), (wk_sb, bk_sb, "k"), (wv_sb, bv_sb, "v"),
                ):
                    ps = qpool.tile([P, TSB], f32, tag="mmq", name="ps")
                    for kt in range(8):
                        nc.tensor.matmul(
                            ps[:],
                            w_sb[:, kt, :],
                            xt[:, kt, :],
                            start=(kt == 0), stop=(kt == 7),
                        )
                    if kind == "q":
                        nc.vector.tensor_scalar_add(
                            qT_sb[:, tsb * TSB:(tsb + 1) * TSB], ps[:], b_sb[:, 0:1])
                    elif kind == "k":
                        nc.vector.tensor_scalar_add(
                            kT_sb[:, tsb * TSB:(tsb + 1) * TSB], ps[:], b_sb[:, 0:1])
                    else:
                        vt_t = vtp.tile([P, TSB], f32, tag="vt", name="vt_t")
                        nc.vector.tensor_scalar_add(vt_t[:], ps[:], b_sb[:, 0:1])
                        for q4 in range(4):
                            pst = ptr.tile([P, P], f32, tag="tr", name="pst")
                            nc.tensor.transpose(
                                pst[:], vt_t[:, q4 * P:(q4 + 1) * P], ident_sb[:])
                            idx = tsb * 4 + q4
                            for h in range(HPC):
                                nc.vector.tensor_copy(
                                    v_sb[:, idx * 130 + h * 65: idx * 130 + h * 65 + 64],
                                    pst[:, h * 64:(h + 1) * 64])
                    yield

            # Fine-grained software-pipelined emission. The per-engine streams
            # are statically ordered by emission order, so interleave: between
            # S-units of superblock (b, sb), drain PV-units of the previous
            # superblock and QKV-units of batch b+1 as PE fill work.
            from collections import deque
            fill = deque()     # (kind, key, generator)

            def drain_one():
                while fill:
                    kind, key, g = fill[0]
                    try:
                        next(g)
                        return True
                    except StopIteration:
                        fill.popleft()
                return False

            def force_drain_qkv(max_tsb):
                # emit any queued QKV units for tsb <= max_tsb (data deps!)
                for ent in list(fill):
                    kind, key, g = ent
                    if kind == "qkv" and key <= max_tsb:
                        for _ in g:
                            pass
                        fill.remove(ent)

            for tsb in range(4):
                for _ in qkv_units(tsb):
                    pass
            sblocks = [(b, sb) for b in range(B) for sb in range(SB_PER_B)]
            for i, (b, sb) in enumerate(sblocks):
                force_drain_qkv(4 * b + sb)
                if b + 1 < B:
                    fill.append(("qkv", 4 * (b + 1) + sb, qkv_units(4 * (b + 1) + sb)))
                pts = {}
                n_s = 2 * (4 * sb + 4)
                sgen = attn_S_units(b, sb, pts)
                cad = CFG.get('cad', 1)
                for k, _ in enumerate(sgen):
                    if (k + 1) % cad == 0:
                        drain_one()
                fill.append(("pv", (b, sb), attn_PV_units(b, sb, pts)))
            while drain_one():
                pass

    nc.compile()
    return nc


class _Runner:
    """Compiles the Bass module once and exposes a sharded 8-core callable."""

    def __init__(self):
        import jax
        import jax.numpy as jnp  # noqa: F401
        from jax.sharding import Mesh, PartitionSpec
        from jax.experimental.shard_map import shard_map
        import concourse.mybir as mybir
        from concourse import bass2jax

        self.jax = jax
        nc = _build_nc()
        self.nc = nc
        bass2jax.install_neuronx_cc_hook()

        partition_name = (nc.partition_id_tensor.name
                          if nc.partition_id_tensor else None)
        in_names, out_names, out_avals, zero_shapes = [], [], [], []
        for alloc in nc.m.functions[0].allocations:
            if not isinstance(alloc, mybir.MemoryLocationSet):
                continue
            name = alloc.memorylocations[0].name
            if alloc.kind == "ExternalInput":
                if name != partition_name:
                    in_names.append(name)
            elif alloc.kind == "ExternalOutput":
                out_names.append(name)
                shape = tuple(alloc.tensor_shape)
                dtype = mybir.dt.np(alloc.dtype)
                out_avals.append(jax.core.ShapedArray(shape, dtype))
                zero_shapes.append((shape, dtype))
        self.in_names = list(in_names)
        self.out_names = list(out_names)
        self.zero_shapes = zero_shapes
        n_params = len(in_names)
        n_outs = len(out_names)
        all_in_names = in_names + out_names
        if partition_name is not None:
            all_in_names = all_in_names + [partition_name]

        def _body(*args):
            operands = list(args)
            if partition_name is not None:
                operands.append(bass2jax.partition_id_tensor())
            outs = bass2jax._bass_exec_p.bind(
                *operands,
                out_avals=tuple(out_avals),
                in_names=tuple(all_in_names),
                out_names=tuple(out_names),
                lowering_input_output_aliases=(),
                sim_require_finite=True,
                sim_require_nnan=True,
                nc=nc,
            )
            return tuple(outs)

        devices = jax.devices()[:N_CORES]
        mesh = Mesh(np.asarray(devices), ("core",))
        self.mesh = mesh
        self.spec = PartitionSpec("core")
        donate = tuple(range(n_params, n_params + n_outs))
        self.sharded = jax.jit(
            shard_map(
                _body, mesh=mesh,
                in_specs=(PartitionSpec("core"),) * (n_params + n_outs),
                out_specs=(PartitionSpec("core"),) * n_outs,
                check_rep=False,
            ),
            donate_argnums=donate,
            keep_unused=True,
        )

    def make_zero_outs(self):
        return [np.zeros((N_CORES * s[0], *s[1:]), d) for s, d in self.zero_shapes]

    def run(self, concat_inputs):
        out_arrs = self.sharded(*concat_inputs, *self.make_zero_outs())
        return [np.asarray(a) for a in out_arrs]


def _get_runner():
    global _RUNNER
    if _RUNNER is None:
        _RUNNER = _Runner()
    return _RUNNER


def prep_inputs(x, Wq, bq, Wk, bk, Wv, bv, Wp, bp):
    """Build the concatenated (axis-0 stacked over cores) device inputs."""
    x = np.asarray(x, np.float32).reshape(BT, C)
    xT = np.ascontiguousarray(x.T)
    import ml_dtypes
    tri01 = np.triu(np.ones((P, P))).astype(ml_dtypes.bfloat16)
    ident = np.eye(P, dtype=np.float32)

    per_core = {n: [] for n in ("xT", "wqT", "wkT", "wvT", "bq", "bk", "bv",
                                "wpT", "tri01", "ident")}
    for i in range(N_CORES):
        cs = slice(i * CH, (i + 1) * CH)
        per_core["xT"].append(xT)
        per_core["wqT"].append(np.ascontiguousarray(np.asarray(Wq, np.float32)[cs, :].T))
        per_core["wkT"].append(np.ascontiguousarray(np.asarray(Wk, np.float32)[cs, :].T))
        per_core["wvT"].append(np.ascontiguousarray(np.asarray(Wv, np.float32)[cs, :].T))
        per_core["bq"].append(np.asarray(bq, np.float32)[cs].reshape(CH, 1))
        per_core["bk"].append(np.asarray(bk, np.float32)[cs].reshape(CH, 1))
        per_core["bv"].append(np.asarray(bv, np.float32)[cs].reshape(CH, 1))
        per_core["wpT"].append(np.ascontiguousarray(np.asarray(Wp, np.float32)[:, cs].T))
        per_core["tri01"].append(tri01)
        per_core["ident"].append(ident)
    return per_core


def kernel(x, Wq, bq, Wk, bk, Wv, bv, Wp, bp):
    runner = _get_runner()
    per_core = prep_inputs(x, Wq, bq, Wk, bk, Wv, bv, Wp, bp)
    concat_in = [np.concatenate(per_core[n], axis=0) for n in runner.in_names]
    outs = runner.run(concat_in)
    # single output: partial [8 * BT, C]
    partials = outs[0].reshape(N_CORES, BT, C)
    out = partials.sum(axis=0) + np.asarray(bp, np.float32)[None, :]
    return out.reshape(B, T, C).astype(np.float32)


# revision 27
# speedup vs baseline: 2.3619x; 2.3619x over previous
"""Causal self-attention (B=4, T=2048, C=1024, H=16) on 8 trn2 NeuronCores.

Sharding: head-parallel. Each core owns 2 of the 16 heads (= 128 of the 1024
qkv channels). QKV projections are column-parallel, attention is fully local
per head, the output projection is row-parallel and the 8 partial outputs are
summed on the host (+ bp).

Device kernel (per core):
  - qT/kT/vT = W_slice^T-form matmuls over xT (fp32r, N=512) -> SBUF resident
  - S^T tiles [s=128, t<=512] = kT^T-slices x qT-slices (fp32r)
  - P^T = exp(S^T/8 + causal_additive_mask) on ACT (no-max softmax; scores
    are O(6) so exp cannot overflow), output bf16
  - P@V + softmax denominator in one bf16 matmul (ones column appended to V)
  - y = num * (1/den) on DVE, PE-transpose to yT, Wp matmul (fp32r) ->
    partial output [8192, 1024] in DRAM
"""

import os
import sys

for _p in ("/opt/trn_rl_repo", "/root/.axon_site/_ro/trn_rl_repo"):
    if os.path.isdir(_p) and _p not in sys.path:
        sys.path.insert(0, _p)

import numpy as np

B, T, C = 4, 2048, 1024
H, D = 16, 64
N_CORES = 8
CH = C // N_CORES          # qkv channels per core (= 2 heads x 64)
HPC = H // N_CORES         # heads per core
BT = B * T                 # 8192 tokens
P = 128
TSB = 512                  # token superblock
N_TSB = BT // TSB          # 16
SB_PER_B = T // TSB        # 4 superblocks per batch
ST_PER_B = T // P          # 16 s-tiles per batch

_RUNNER = None
CFG = {'pm': 4, 'pmq': 2, 'ptr': 1, 'ppv': 1, 'pt': 60, 'pipe': 1, 'cad': 3}


def _build_nc():
    import concourse.mybir as mybir
    import concourse.tile as tile
    from concourse import bacc

    f32 = mybir.dt.float32
    f32r = mybir.dt.float32r
    bf16 = mybir.dt.bfloat16
    MULT = mybir.AluOpType.mult
    EXP = mybir.ActivationFunctionType.Exp

    nc = bacc.Bacc("TRN2", target_bir_lowering=False, debug=False,
                   num_devices=N_CORES)

    xT = nc.dram_tensor("xT", [C, BT], f32r, kind="ExternalInput")
    wqT = nc.dram_tensor("wqT", [C, CH], f32r, kind="ExternalInput")
    wkT = nc.dram_tensor("wkT", [C, CH], f32r, kind="ExternalInput")
    wvT = nc.dram_tensor("wvT", [C, CH], f32r, kind="ExternalInput")
    bq = nc.dram_tensor("bq", [CH, 1], f32, kind="ExternalInput")
    bk = nc.dram_tensor("bk", [CH, 1], f32, kind="ExternalInput")
    bv = nc.dram_tensor("bv", [CH, 1], f32, kind="ExternalInput")
    wpT = nc.dram_tensor("wpT", [CH, C], f32r, kind="ExternalInput")
    tri01 = nc.dram_tensor("tri01", [P, P], bf16, kind="ExternalInput")
    ident = nc.dram_tensor("ident", [P, P], f32, kind="ExternalInput")
    outp = nc.dram_tensor("outp", [BT, C], f32, kind="ExternalOutput")

    with tile.TileContext(nc) as tc:
        with (
            tc.tile_pool(name="const", bufs=1) as const,
            tc.tile_pool(name="big", bufs=1) as big,
            tc.tile_pool(name="xp", bufs=CFG.get('xp', 2)) as xp,
            tc.tile_pool(name="vt", bufs=CFG.get('vt', 2)) as vtp,
            tc.tile_pool(name="pt", bufs=CFG['pt']) as ptp,
            tc.tile_pool(name="yp", bufs=CFG.get('yp', 3)) as yp,
            tc.tile_pool(name="ytp", bufs=CFG.get('ytp', 2)) as ytp,
            tc.tile_pool(name="rp", bufs=4) as rp,
            tc.tile_pool(name="op", bufs=CFG.get('op', 3)) as op,
            tc.tile_pool(name="pm", bufs=CFG['pm'], space="PSUM") as pm,
            tc.tile_pool(name="pmq", bufs=max(CFG.get('pmq', 0), 1), space="PSUM") as pmq,
            tc.tile_pool(name="ptr", bufs=CFG['ptr'], space="PSUM") as ptr,
            tc.tile_pool(name="ppv", bufs=CFG['ppv'], space="PSUM") as ppv,
        ):
            # ---- constants (wq first: it gates the first matmul) ----
            wq_sb = const.tile([P, 8, CH], f32r, tag="wq")
            wk_sb = const.tile([P, 8, CH], f32r, tag="wk")
            wv_sb = const.tile([P, 8, CH], f32r, tag="wv")
            nc.sync.dma_start(wq_sb[:, 0:2, :],
                              wqT.ap().rearrange("(a p) m -> p a m", p=P)[:, 0:2, :])
            nc.sync.dma_start(wq_sb[:, 2:8, :],
                              wqT.ap().rearrange("(a p) m -> p a m", p=P)[:, 2:8, :])
            nc.sync.dma_start(wk_sb[:], wkT.ap().rearrange("(a p) m -> p a m", p=P))
            nc.sync.dma_start(wv_sb[:], wvT.ap().rearrange("(a p) m -> p a m", p=P))
            wp_sb = const.tile([P, C], f32r, tag="wp")
            bq_sb = const.tile([P, 1], f32, tag="bq")
            bk_sb = const.tile([P, 1], f32, tag="bk")
            bv_sb = const.tile([P, 1], f32, tag="bv")
            mask_sb = const.tile([P, P], bf16, tag="mask")
            ident_sb = const.tile([P, P], f32, tag="ident")
            nc.sync.dma_start(bq_sb[:], bq.ap())
            nc.sync.dma_start(bk_sb[:], bk.ap())
            nc.sync.dma_start(bv_sb[:], bv.ap())
            nc.sync.dma_start(wp_sb[:], wpT.ap())
            nc.sync.dma_start(mask_sb[:], tri01.ap())
            nc.sync.dma_start(ident_sb[:], ident.ap())

            # ---- resident activation buffers ----
            qT_sb = big.tile([P, BT], f32r, tag="qT")
            kT_sb = big.tile([P, BT], f32r, tag="kT")
            # v layout: per global s-tile idx (64 of them): 130 bf16 cols =
            # [head0 v (64) | ones | head1 v (64) | ones]
            v_sb = big.tile([P, 64 * 130], bf16, tag="v")
            nc.vector.memset(v_sb[:, 64::65], 1.0)

            def attn_S_units(b, sb, pts):
                """Generator: one (h, st) score tile + exp per unit."""
                t0 = b * T + sb * TSB
                nst = 4 * sb + 4
                for h in range(HPC):
                    hs = h * 64
                    for st in range(nst):
                        j0 = max(0, st - 4 * sb)
                        n0 = j0 * P
                        ps = pm.tile([P, TSB], f32, tag="mm", name="ps")
                        lhs = kT_sb[hs:hs + 64,
                                    (b * ST_PER_B + st) * P:(b * ST_PER_B + st + 1) * P]
                        rhs = qT_sb[hs:hs + 64, t0 + n0: t0 + TSB]
                        nc.tensor.matmul(
                            ps[:, n0:TSB], lhs, rhs,
                            start=True, stop=True)
                        ptile = ptp.tile([P, TSB], bf16, tag="pt", name="ptile")
                        nc.scalar.activation(
                            ptile[:, n0:TSB], ps[:, n0:TSB], EXP, scale=0.125)
                        if st >= 4 * sb:
                            nc.vector.tensor_tensor(
                                ptile[:, n0:n0 + P], ptile[:, n0:n0 + P],
                                mask_sb[:], MULT)
                        pts[(h, st)] = ptile
                        yield

            def attn_PV_units(b, sb, pts):
                """Generator: one output t-block (PV + div + transpose + Wp)."""
                for j in range(4):
                    y_t = yp.tile([P, P], f32, tag="y", name="y_t")
                    for h in range(HPC):
                        pv = ppv.tile([P, 65], f32, tag="pv", name="pv")
                        nv = 4 * sb + j + 1
                        for st in range(nv):
                            ptile = pts[(h, st)]
                            idx = b * ST_PER_B + st
                            nc.tensor.matmul(
                                pv[:],
                                ptile[:, j * P:(j + 1) * P],
                                v_sb[:, idx * 130 + h * 65: idx * 130 + h * 65 + 65],
                                start=(st == 0), stop=(st == nv - 1))
                        rec = rp.tile([P, 1], f32, tag="rec", name="rec")
                        nc.vector.reciprocal(rec[:], pv[:, 64:65])
                        nc.vector.tensor_scalar_mul(
                            y_t[:, h * 64:(h + 1) * 64], pv[:, 0:64], rec[:, 0:1])
                    pst = ptr.tile([P, P], f32, tag="tr", name="pst")
                    nc.tensor.transpose(pst[:], y_t[:], ident_sb[:])
                    yt_t = ytp.tile([P, P], f32r, tag="yt", name="yt_t")
                    nc.vector.tensor_copy(yt_t[:], pst[:])
                    r = (b * SB_PER_B + sb) * 4 + j
                    ot = op.tile([P, C], f32, tag="ot", name="ot")
                    for half in range(2):
                        wps = pm.tile([P, TSB], f32, tag="mm", name="wps")
                        nc.tensor.matmul(
                            wps[:], yt_t[:], wp_sb[:, half * TSB:(half + 1) * TSB],
                            start=True, stop=True)
                        nc.any.tensor_copy(
                            ot[:, half * TSB:(half + 1) * TSB], wps[:])
                    nc.sync.dma_start(outp.ap()[r * P:(r + 1) * P, :], ot[:])
                    yield

            def qkv_units(tsb):
                """Generator: one projection per unit (DMA rides with unit 0)."""
                xt = xp.tile([P, 8, TSB], f32r, tag="xt", name="xt")
                src = xT.ap().rearrange("(a p) t -> p a t", p=P)
                for c in range(4):  # chunked so matmuls start on first k-tiles
                    nc.sync.dma_start(
                        xt[:, 2 * c:2 * c + 2, :],
                        src[:, 2 * c:2 * c + 2, tsb * TSB:(tsb + 1) * TSB])
                for unit in qkv_superblock_body(tsb, xt):
                    yield

            def qkv_superblock_body(tsb, xt):
                qpool = pmq if CFG.get('pmq', 0) else pm
                for (w_sb, b_sb, kind) in (
                    (wq_sb, bq_sb, "q"), (wk_sb, bk_sb, "k"), (wv_sb, bv_sb, "v"),
                ):
                    ps = qpool.tile([P, TSB], f32, tag="mmq", name="ps")
                    for kt in range(8):
                        nc.tensor.matmul(
                            ps[:],
                            w_sb[:, kt, :],
                            xt[:, kt, :],
                            start=(kt == 0), stop=(kt == 7),
                        )
                    if kind == "q":
                        nc.vector.tensor_scalar_add(
                            qT_sb[:, tsb * TSB:(tsb + 1) * TSB], ps[:], b_sb[:, 0:1])
                    elif kind == "k":
                        nc.vector.tensor_scalar_add(
                            kT_sb[:, tsb * TSB:(tsb + 1) * TSB], ps[:], b_sb[:, 0:1])
                    else:
                        vt_t = vtp.tile([P, TSB], f32, tag="vt", name="vt_t")
                        nc.vector.tensor_scalar_add(vt_t[:], ps[:], b_sb[:, 0:1])
                        for q4 in range(4):
                            pst = ptr.tile([P, P], f32, tag="tr", name="pst")
                            nc.tensor.transpose(
                                pst[:], vt_t[:, q4 * P:(q4 + 1) * P], ident_sb[:])
                            idx = tsb * 4 + q4
                            for h in range(HPC):
                                nc.vector.tensor_copy(
                                    v_sb[:, idx * 130 + h * 65: idx * 130 + h * 65 + 64],
                                    pst[:, h * 64:(h + 1) * 64])
                    yield

            # Fine-grained software-pipelined emission. The per-engine streams
            # are statically ordered by emission order, so interleave: between
            # S-units of superblock (b, sb), drain PV-units of the previous
            # superblock and QKV-units of batch b+1 as PE fill work.
            from collections import deque
            fill = deque()     # (kind, key, generator)

            def drain_one():
                while fill:
                    kind, key, g = fill[0]
                    try:
                        next(g)
                        return True
                    except StopIteration:
                        fill.popleft()
                return False

            def force_drain_qkv(max_tsb):
                # emit any queued QKV units for tsb <= max_tsb (data deps!)
                for ent in list(fill):
                    kind, key, g = ent
                    if kind == "qkv" and key <= max_tsb:
                        for _ in g:
                            pass
                        fill.remove(ent)

            for tsb in range(4):
                for _ in qkv_units(tsb):
                    pass
            sblocks = [(b, sb) for b in range(B) for sb in range(SB_PER_B)]
            for i, (b, sb) in enumerate(sblocks):
                force_drain_qkv(4 * b + sb)
                if b + 1 < B:
                    fill.append(("qkv", 4 * (b + 1) + sb, qkv_units(4 * (b + 1) + sb)))
                pts = {}
                n_s = 2 * (4 * sb + 4)
                sgen = attn_S_units(b, sb, pts)
                cad = CFG.get('cad', 1)
                if cad == 0:  # adaptive: spread ~8 fill units across S units
                    cad = max(1, n_s // 8)
                for k, _ in enumerate(sgen):
                    if (k + 1) % cad == 0:
                        drain_one()
                fill.append(("pv", (b, sb), attn_PV_units(b, sb, pts)))
            while drain_one():
                pass

    nc.compile()
    return nc


class _Runner:
    """Compiles the Bass module once and exposes a sharded 8-core callable."""

    def __init__(self):
        import jax
        import jax.numpy as jnp  # noqa: F401
        from jax.sharding import Mesh, PartitionSpec
        from jax.experimental.shard_map import shard_map
        import concourse.mybir as mybir
        from concourse import bass2jax

        self.jax = jax
        nc = _build_nc()
        self.nc = nc
        bass2jax.install_neuronx_cc_hook()

        partition_name = (nc.partition_id_tensor.name
                          if nc.partition_id_tensor else None)
        in_names, out_names, out_avals, zero_shapes = [], [], [], []
        for alloc in nc.m.functions[0].allocations:
            if not isinstance(alloc, mybir.MemoryLocationSet):
                continue
            name = alloc.memorylocations[0].name
            if alloc.kind == "ExternalInput":
                if name != partition_name:
                    in_names.append(name)
            elif alloc.kind == "ExternalOutput":
                out_names.append(name)
                shape = tuple(alloc.tensor_shape)
                dtype = mybir.dt.np(alloc.dtype)
                out_avals.append(jax.core.ShapedArray(shape, dtype))
                zero_shapes.append((shape, dtype))
        self.in_names = list(in_names)
        self.out_names = list(out_names)
        self.zero_shapes = zero_shapes
        n_params = len(in_names)
        n_outs = len(out_names)
        all_in_names = in_names + out_names
        if partition_name is not None:
            all_in_names = all_in_names + [partition_name]

        def _body(*args):
            operands = list(args)
            if partition_name is not None:
                operands.append(bass2jax.partition_id_tensor())
            outs = bass2jax._bass_exec_p.bind(
                *operands,
                out_avals=tuple(out_avals),
                in_names=tuple(all_in_names),
                out_names=tuple(out_names),
                lowering_input_output_aliases=(),
                sim_require_finite=True,
                sim_require_nnan=True,
                nc=nc,
            )
            return tuple(outs)

        devices = jax.devices()[:N_CORES]
        mesh = Mesh(np.asarray(devices), ("core",))
        self.mesh = mesh
        self.spec = PartitionSpec("core")
        donate = tuple(range(n_params, n_params + n_outs))
        self.sharded = jax.jit(
            shard_map(
                _body, mesh=mesh,
                in_specs=(PartitionSpec("core"),) * (n_params + n_outs),
                out_specs=(PartitionSpec("core"),) * n_outs,
                check_rep=False,
            ),
            donate_argnums=donate,
            keep_unused=True,
        )

    def make_zero_outs(self):
        return [np.zeros((N_CORES * s[0], *s[1:]), d) for s, d in self.zero_shapes]

    def run(self, concat_inputs):
        out_arrs = self.sharded(*concat_inputs, *self.make_zero_outs())
        return [np.asarray(a) for a in out_arrs]


def _get_runner():
    global _RUNNER
    if _RUNNER is None:
        _RUNNER = _Runner()
    return _RUNNER


def prep_inputs(x, Wq, bq, Wk, bk, Wv, bv, Wp, bp):
    """Build the concatenated (axis-0 stacked over cores) device inputs."""
    x = np.asarray(x, np.float32).reshape(BT, C)
    xT = np.ascontiguousarray(x.T)
    import ml_dtypes
    tri01 = np.triu(np.ones((P, P))).astype(ml_dtypes.bfloat16)
    ident = np.eye(P, dtype=np.float32)

    per_core = {n: [] for n in ("xT", "wqT", "wkT", "wvT", "bq", "bk", "bv",
                                "wpT", "tri01", "ident")}
    for i in range(N_CORES):
        cs = slice(i * CH, (i + 1) * CH)
        per_core["xT"].append(xT)
        per_core["wqT"].append(np.ascontiguousarray(np.asarray(Wq, np.float32)[cs, :].T))
        per_core["wkT"].append(np.ascontiguousarray(np.asarray(Wk, np.float32)[cs, :].T))
        per_core["wvT"].append(np.ascontiguousarray(np.asarray(Wv, np.float32)[cs, :].T))
        per_core["bq"].append(np.asarray(bq, np.float32)[cs].reshape(CH, 1))
        per_core["bk"].append(np.asarray(bk, np.float32)[cs].reshape(CH, 1))
        per_core["bv"].append(np.asarray(bv, np.float32)[cs].reshape(CH, 1))
        per_core["wpT"].append(np.ascontiguousarray(np.asarray(Wp, np.float32)[:, cs].T))
        per_core["tri01"].append(tri01)
        per_core["ident"].append(ident)
    return per_core


def kernel(x, Wq, bq, Wk, bk, Wv, bv, Wp, bp):
    runner = _get_runner()
    per_core = prep_inputs(x, Wq, bq, Wk, bk, Wv, bv, Wp, bp)
    concat_in = [np.concatenate(per_core[n], axis=0) for n in runner.in_names]
    outs = runner.run(concat_in)
    # single output: partial [8 * BT, C]
    partials = outs[0].reshape(N_CORES, BT, C)
    out = partials.sum(axis=0) + np.asarray(bp, np.float32)[None, :]
    return out.reshape(B, T, C).astype(np.float32)
